# revision 13
# baseline (speedup 1.0000x reference)
"""AttentionFusionNet2 Trainium2 kernel — 8-core data-parallel over batch.

Per core (2 batch elements):
  Phase 1 (parallel over all timesteps, feature-major layouts):
    projections a/v/l, attention-fusion context+softmax, fusion, and
    gates_pre = Wih @ fusion + (bih+bhh)  -> DRAM stream [16,128,BL,S]
  Phase 2 (sequential LSTM, 2048 steps):
    gates.T = Whh @ h.T via 64 matmuls/step (Whh stationary, bf16 for FWL),
    gates land in 4 PSUM banks [128, 4*BL] packed (gate-row-chunk, batch),
    elementwise c/h update full-width on DVE/ACT, h written straight into
    a bf16 ring in the rhs layout for the next step's matmul.
All input/weight layout massaging is done host-side in numpy.
"""
import sys
for _p in ('/opt/trn_rl_repo', '/root/problem'):
    if _p not in sys.path:
        sys.path.insert(0, _p)

import numpy as np
import ml_dtypes

B, S_FULL, A, V, L, H = 16, 2048, 512, 768, 1024, 512
NC = 8
BL = B // NC          # batch per core
GD = 4 * H            # 2048 gate rows
MD_G = GD // 128      # 16 gate m-tiles
KD_H = H // 128       # 4 h k-chunks

_CACHE = {}


def _lhsT_tiles(W, dtype=np.float32):
    """W [M*128 out, K*128 in] -> sbuf layout [128, M*K*128],
    tile i=(m*K+k) at cols i*128:(i+1)*128 with tile[p,c] = W[m*128+c, k*128+p]."""
    MR, KR = W.shape
    MD, KD = MR // 128, KR // 128
    t = W.reshape(MD, 128, KD, 128).transpose(3, 0, 2, 1)
    return np.ascontiguousarray(t.reshape(128, MD * KD * 128)).astype(dtype)


def _build(S, T):
    import concourse.bass as bass
    import concourse.tile as tile
    from concourse import mybir
    import tile_patch
    tile_patch.install()

    f32 = mybir.dt.float32
    bf16 = mybir.dt.bfloat16
    CH = min(256, S)               # phase-1 column chunk (must divide S)
    NCH = (BL * S) // CH
    SC = S // T                    # phase-2 chunks
    BLT = BL * T

    nc = bass.Bass(target_bir_lowering=False)
    P = lambda n, shp, dt=f32: nc.declare_dram_parameter(n, shp, dt, isOutput=False)
    aT = P("aT", [A, BL * S])
    vT = P("vT", [V, BL * S])
    lT = P("lT", [L, BL * S])
    h0p = P("h0p", [128, KD_H * BL])
    c0p = P("c0p", [128, KD_H * BL])
    waT = P("waT", [128, (H // 128) * (A // 128) * 128])
    wvT = P("wvT", [128, (H // 128) * (V // 128) * 128])
    wlT = P("wlT", [128, (H // 128) * (L // 128) * 128])
    wcT = P("wcT", [128, (H // 128) * ((3 * H) // 128) * 128])
    wihT = P("wihT", [128, MD_G * KD_H * 128])
    whhT = P("whhT", [128, MD_G * KD_H * 128], bf16)
    ones_row = P("ones_row", [1, 512])
    ones_col = P("ones_col", [128, 1])
    brow_a = P("brow_a", [1, H])
    brow_v = P("brow_v", [1, H])
    brow_l = P("brow_l", [1, H])
    brow_c = P("brow_c", [1, H])
    brow_g = P("brow_g", [1, GD])

    out_ext = nc.declare_dram_parameter("out", [KD_H, 128, BL, S], f32, isOutput=True)
    hout = nc.declare_dram_parameter("hout", [128, KD_H * BL], f32, isOutput=True)
    cout = nc.declare_dram_parameter("cout", [128, KD_H * BL], f32, isOutput=True)

    Sig = mybir.ActivationFunctionType.Sigmoid
    Tanh = mybir.ActivationFunctionType.Tanh
    Exp = mybir.ActivationFunctionType.Exp

    with tile.TileContext(nc) as tc:
        with tc.tile_pool(name="dram", bufs=1, space="DRAM") as dpool:
            gp_dram = dpool.tile([MD_G, 128, BL, S], f32)

            # ---------------- Phase 1 ----------------
            with (
                tc.tile_pool(name="w1", bufs=1) as w1,
                tc.tile_pool(name="io1", bufs=2) as io1,
                tc.tile_pool(name="sb1", bufs=1) as sb1,
                tc.tile_pool(name="sbs", bufs=4) as sbs,
                tc.tile_pool(name="gpc", bufs=4) as gpc,
                tc.tile_pool(name="ps1", bufs=3, space="PSUM") as ps1,
                tc.tile_pool(name="psc", bufs=3, space="PSUM") as psc,
                tc.tile_pool(name="row1", bufs=1) as row1,
            ):
                wa_sb = w1.tile(waT.shape, f32)
                nc.gpsimd.dma_start(wa_sb[:], waT[:])
                wv_sb = w1.tile(wvT.shape, f32)
                nc.gpsimd.dma_start(wv_sb[:], wvT[:])
                wl_sb = w1.tile(wlT.shape, f32)
                nc.gpsimd.dma_start(wl_sb[:], wlT[:])
                wc_sb = w1.tile(wcT.shape, f32)
                nc.gpsimd.dma_start(wc_sb[:], wcT[:])
                wih_sb = w1.tile(wihT.shape, f32)
                nc.gpsimd.dma_start(wih_sb[:], wihT[:])
                ones_sb = w1.tile([1, 512], f32)
                nc.gpsimd.dma_start(ones_sb[:], ones_row[:])
                onescol_sb = w1.tile([128, 1], f32)
                nc.gpsimd.dma_start(onescol_sb[:], ones_col[:])
                br_a = w1.tile([1, H], f32)
                nc.gpsimd.dma_start(br_a[:], brow_a[:])
                br_v = w1.tile([1, H], f32)
                nc.gpsimd.dma_start(br_v[:], brow_v[:])
                br_l = w1.tile([1, H], f32)
                nc.gpsimd.dma_start(br_l[:], brow_l[:])
                br_c = w1.tile([1, H], f32)
                nc.gpsimd.dma_start(br_c[:], brow_c[:])
                br_g = w1.tile([1, GD], f32)
                nc.gpsimd.dma_start(br_g[:], brow_g[:])

                def tiles(w_sb, m, KD):
                    return [w_sb[:, (m * KD + k) * 128:(m * KD + k + 1) * 128]
                            for k in range(KD)]

                for cc in range(NCH):
                    c0 = cc * CH
                    # inputs for this chunk, feature-major [128,(rt,CH)]
                    at_t = io1.tile([128, (A // 128) * CH], f32, tag="at")
                    nc.gpsimd.dma_start(
                        at_t[:].rearrange("p (r c) -> p r c", r=A // 128),
                        aT[:, c0:c0 + CH].rearrange("(r p) c -> p r c", p=128))
                    vt_t = io1.tile([128, (V // 128) * CH], f32, tag="vt")
                    nc.gpsimd.dma_start(
                        vt_t[:].rearrange("p (r c) -> p r c", r=V // 128),
                        vT[:, c0:c0 + CH].rearrange("(r p) c -> p r c", p=128))
                    lt_t = io1.tile([128, (L // 128) * CH], f32, tag="lt")
                    nc.gpsimd.dma_start(
                        lt_t[:].rearrange("p (r c) -> p r c", r=L // 128),
                        lT[:, c0:c0 + CH].rearrange("(r p) c -> p r c", p=128))

                    def proj(w_sb, in_t, KD, brow, out_sb):
                        for m in range(H // 128):
                            ps = ps1.tile([128, CH], f32, tag="pp")
                            wt = tiles(w_sb, m, KD)
                            for k in range(KD):
                                nc.tensor.matmul(
                                    ps[:], wt[k], in_t[:, k * CH:(k + 1) * CH],
                                    start=(k == 0), stop=False)
                            nc.tensor.matmul(
                                ps[:], brow[0:1, m * 128:(m + 1) * 128],
                                ones_sb[0:1, 0:CH], start=False, stop=True)
                            nc.vector.tensor_copy(
                                out_sb[:, m * CH:(m + 1) * CH], ps[:])

                    ap_sb = sb1.tile([128, (H // 128) * CH], f32, tag="ap")
                    proj(wa_sb, at_t, A // 128, br_a, ap_sb)
                    vp_sb = sb1.tile([128, (H // 128) * CH], f32, tag="vp")
                    proj(wv_sb, vt_t, V // 128, br_v, vp_sb)
                    lp_sb = sb1.tile([128, (H // 128) * CH], f32, tag="lp")
                    proj(wl_sb, lt_t, L // 128, br_l, lp_sb)

                    # context = tanh(Wc @ [aP;vP;lP] + bc)
                    ctx_sb = sb1.tile([128, (H // 128) * CH], f32, tag="ctx")
                    cat = [ap_sb, vp_sb, lp_sb]
                    for m in range(H // 128):
                        ps = ps1.tile([128, CH], f32, tag="pp")
                        wt = tiles(wc_sb, m, (3 * H) // 128)
                        for k in range(12):
                            nc.tensor.matmul(
                                ps[:], wt[k],
                                cat[k // 4][:, (k % 4) * CH:(k % 4 + 1) * CH],
                                start=(k == 0), stop=False)
                        nc.tensor.matmul(
                            ps[:], br_c[0:1, m * 128:(m + 1) * 128],
                            ones_sb[0:1, 0:CH], start=False, stop=True)
                        nc.scalar.activation(
                            ctx_sb[:, m * CH:(m + 1) * CH], ps[:], Tanh)

                    # scores: s_m = sum_h (mP * ctx), via ones-column reduce
                    sc_ps = []
                    for mp in (ap_sb, vp_sb, lp_sb):
                        sp = psc.tile([1, CH], f32, tag="sc")
                        for k in range(H // 128):
                            pr = sbs.tile([128, CH], f32, tag="prod")
                            nc.vector.tensor_mul(
                                pr[:], mp[:, k * CH:(k + 1) * CH],
                                ctx_sb[:, k * CH:(k + 1) * CH])
                            nc.tensor.matmul(
                                sp[:], onescol_sb[:, 0:1],
                                pr[:], start=(k == 0), stop=(k == 3))
                        sc_ps.append(sp)

                    e_sb = []
                    for sp in sc_ps:
                        e = sbs.tile([1, CH], f32, tag="esb")
                        nc.scalar.activation(e[:], sp[:], Exp)
                        e_sb.append(e)
                    ssum = sbs.tile([1, CH], f32, tag="ssum")
                    nc.vector.tensor_add(ssum[:], e_sb[0][:], e_sb[1][:])
                    nc.vector.tensor_add(ssum[:], ssum[:], e_sb[2][:])
                    rinv = sbs.tile([1, CH], f32, tag="rinv")
                    nc.vector.reciprocal(rinv[:], ssum[:])
                    w_sb = []
                    for e in e_sb:
                        w = sbs.tile([1, CH], f32, tag="wsb")
                        nc.vector.tensor_mul(w[:], e[:], rinv[:])
                        w_sb.append(w)

                    # broadcast weights to 128 partitions (outer product)
                    wB = []
                    for w in w_sb:
                        bp = ps1.tile([128, CH], f32, tag="pp")
                        nc.tensor.matmul(bp[:], ones_sb[0:1, 0:128], w[:],
                                         start=True, stop=True)
                        wb = sbs.tile([128, CH], f32, tag="wB")
                        nc.vector.tensor_copy(wb[:], bp[:])
                        wB.append(wb)

                    # fusion = aP*wa + vP*wv + lP*wl
                    fus_sb = sb1.tile([128, (H // 128) * CH], f32, tag="fus")
                    for k in range(H // 128):
                        sl = slice(k * CH, (k + 1) * CH)
                        m1 = sbs.tile([128, CH], f32, tag="fm1")
                        nc.vector.tensor_mul(m1[:], ap_sb[:, sl], wB[0][:])
                        m2 = sbs.tile([128, CH], f32, tag="fm2")
                        nc.vector.tensor_mul(m2[:], vp_sb[:, sl], wB[1][:])
                        nc.vector.tensor_add(m1[:], m1[:], m2[:])
                        nc.vector.tensor_mul(m2[:], lp_sb[:, sl], wB[2][:])
                        nc.vector.tensor_add(fus_sb[:, sl], m1[:], m2[:])

                    # gates_pre = Wih @ fusion + (bih+bhh)
                    bI = c0 // S
                    t0 = c0 % S
                    for m in range(MD_G):
                        ps = ps1.tile([128, CH], f32, tag="pp")
                        wt = tiles(wih_sb, m, KD_H)
                        for k in range(KD_H):
                            nc.tensor.matmul(
                                ps[:], wt[k], fus_sb[:, k * CH:(k + 1) * CH],
                                start=(k == 0), stop=False)
                        nc.tensor.matmul(
                            ps[:], br_g[0:1, m * 128:(m + 1) * 128],
                            ones_sb[0:1, 0:CH], start=False, stop=True)
                        g_sb = gpc.tile([128, CH], f32, tag="gp")
                        nc.vector.tensor_copy(g_sb[:], ps[:])
                        nc.gpsimd.dma_start(gp_dram[m, :, bI, t0:t0 + CH], g_sb[:])

            # ---------------- Phase 2: LSTM recurrence ----------------
            with (
                tc.tile_pool(name="w2", bufs=1) as w2,
                tc.tile_pool(name="gpp", bufs=2) as gpp,
                tc.tile_pool(name="ring", bufs=2) as ringp,
                tc.tile_pool(name="st", bufs=8) as stp,
                tc.tile_pool(name="cst", bufs=3) as cstp,
                tc.tile_pool(name="ps2", bufs=8, space="PSUM") as ps2,
            ):
                whh_sb = w2.tile(whhT.shape, bf16)
                nc.gpsimd.dma_start(whh_sb[:], whhT[:])
                h0_sb = w2.tile([128, KD_H * BL], f32)
                nc.gpsimd.dma_start(h0_sb[:], h0p[:])
                c0_sb = w2.tile([128, KD_H * BL], f32)
                nc.gpsimd.dma_start(c0_sb[:], c0p[:])
                h0b = w2.tile([128, KD_H * BL], bf16)
                nc.vector.tensor_copy(h0b[:], h0_sb[:])

                c_prev = c0_sb
                ring_prev = None
                for sc in range(SC):
                    t0 = sc * T
                    gp_sb = gpp.tile([128, MD_G * BL * T], f32, tag="gp2")
                    for bb in range(BL):
                        nc.gpsimd.dma_start(
                            gp_sb[:].rearrange("p (m b t) -> p m b t",
                                               m=MD_G, b=BL)[:, :, bb, :],
                            gp_dram[:, :, bb, t0:t0 + T].rearrange(
                                "m p t -> p m t"))
                    gpv = gp_sb[:].rearrange("p (m b t) -> p m b t", m=MD_G, b=BL)

                    ring = ringp.tile([128, KD_H * BL * T], bf16, tag="ring")
                    rv = ring[:].rearrange("p (k b t) -> p k b t", k=KD_H, b=BL)

                    for t in range(T):
                        if t == 0 and sc == 0:
                            hp = h0b[:].rearrange("p (k b) -> p k b", k=KD_H)
                            rhs = lambda k: hp[:, k, :]
                        elif t == 0:
                            rvp = ring_prev[:].rearrange(
                                "p (k b t) -> p k b t", k=KD_H, b=BL)
                            rhs = lambda k: rvp[:, k, :, T - 1]
                        else:
                            rhs = lambda k: rv[:, k, :, t - 1]

                        pss = []
                        for gt in range(4):
                            ps = ps2.tile([128, KD_H * BL], f32, tag="ps")
                            for j in range(KD_H):
                                m = gt * 4 + j
                                for k in range(KD_H):
                                    nc.tensor.matmul(
                                        ps[:, j * BL:(j + 1) * BL],
                                        whh_sb[:, (m * KD_H + k) * 128:
                                               (m * KD_H + k + 1) * 128],
                                        rhs(k),
                                        start=(k == 0), stop=(k == KD_H - 1))
                            pss.append(ps)

                        acts = []
                        for gt, fn in ((0, Sig), (1, Sig), (2, Tanh), (3, Sig)):
                            gs = stp.tile([128, KD_H * BL], f32, tag="gsb")
                            nc.vector.tensor_add(
                                gs[:].rearrange("p (j b) -> p j b", j=KD_H),
                                pss[gt][:].rearrange("p (j b) -> p j b", j=KD_H),
                                gpv[:, gt * 4:(gt + 1) * 4, :, t])
                            av = stp.tile([128, KD_H * BL], f32, tag="act")
                            nc.scalar.activation(av[:], gs[:], fn)
                            acts.append(av)

                        ia, fa, ga, oa = acts
                        t1 = stp.tile([128, KD_H * BL], f32, tag="t1")
                        nc.vector.tensor_mul(t1[:], ia[:], ga[:])
                        t2 = stp.tile([128, KD_H * BL], f32, tag="t2")
                        nc.vector.tensor_mul(t2[:], fa[:], c_prev[:])
                        c_new = cstp.tile([128, KD_H * BL], f32, tag="c")
                        nc.vector.tensor_add(c_new[:], t1[:], t2[:])
                        tc_t = stp.tile([128, KD_H * BL], f32, tag="tc")
                        nc.scalar.activation(tc_t[:], c_new[:], Tanh)
                        nc.vector.tensor_mul(
                            rv[:, :, :, t],
                            oa[:].rearrange("p (k b) -> p k b", k=KD_H),
                            tc_t[:].rearrange("p (k b) -> p k b", k=KD_H))
                        c_prev = c_new

                    for bb in range(BL):
                        nc.gpsimd.dma_start(
                            out_ext[:, :, bb, t0:t0 + T].rearrange(
                                "k p t -> p k t"),
                            rv[:, :, bb, :])
                    ring_prev = ring

                nc.gpsimd.dma_start(
                    hout[:].rearrange("p (k b) -> p k b", k=KD_H),
                    rv[:, :, :, T - 1])
                nc.gpsimd.dma_start(cout[:], c_prev[:])

    ns = tile_patch.split_multiwait(nc)
    print(f"split {ns} multi-wait instructions", flush=True)
    return nc


def _prep(inputs, S):
    """Host-side prep: per-core in_maps."""
    a_in = np.asarray(inputs["a_input"], np.float32)[:, :S]
    v_in = np.asarray(inputs["v_input"], np.float32)[:, :S]
    l_in = np.asarray(inputs["l_input"], np.float32)[:, :S]
    h0 = np.asarray(inputs["h0"], np.float32)
    c0 = np.asarray(inputs["c0"], np.float32)

    shared = {
        "waT": _lhsT_tiles(np.asarray(inputs["Wa"], np.float32)),
        "wvT": _lhsT_tiles(np.asarray(inputs["Wv"], np.float32)),
        "wlT": _lhsT_tiles(np.asarray(inputs["Wl"], np.float32)),
        "wcT": _lhsT_tiles(np.asarray(inputs["Wc"], np.float32)),
        "wihT": _lhsT_tiles(np.asarray(inputs["Wih"], np.float32)),
        "whhT": _lhsT_tiles(np.asarray(inputs["Whh"], np.float32),
                            ml_dtypes.bfloat16),
        "ones_row": np.ones((1, 512), np.float32),
        "ones_col": np.ones((128, 1), np.float32),
        "brow_a": np.asarray(inputs["ba"], np.float32)[None, :],
        "brow_v": np.asarray(inputs["bv"], np.float32)[None, :],
        "brow_l": np.asarray(inputs["bl"], np.float32)[None, :],
        "brow_c": np.asarray(inputs["bc"], np.float32)[None, :],
        "brow_g": (np.asarray(inputs["bih"], np.float32)
                   + np.asarray(inputs["bhh"], np.float32))[None, :],
    }

    def pack_state(x, b0):          # [B,H] slice -> [128, KD_H*BL]
        sl = x[b0:b0 + BL]          # [BL, H]
        return np.ascontiguousarray(
            sl.reshape(BL, KD_H, 128).transpose(2, 1, 0).reshape(128, KD_H * BL)
        ).astype(np.float32)

    in_maps = []
    for c in range(NC):
        b0 = c * BL
        m = dict(shared)
        m["aT"] = np.ascontiguousarray(
            a_in[b0:b0 + BL].transpose(2, 0, 1).reshape(A, BL * S))
        m["vT"] = np.ascontiguousarray(
            v_in[b0:b0 + BL].transpose(2, 0, 1).reshape(V, BL * S))
        m["lT"] = np.ascontiguousarray(
            l_in[b0:b0 + BL].transpose(2, 0, 1).reshape(L, BL * S))
        m["h0p"] = pack_state(h0[0], b0)
        m["c0p"] = pack_state(c0[0], b0)
        in_maps.append(m)
    return in_maps


def _gather(results, S):
    r_out = np.empty((B, S, H), np.float32)
    hT = np.empty((1, B, H), np.float32)
    cT = np.empty((1, B, H), np.float32)
    for c in range(NC):
        b0 = c * BL
        o = results[c]["out"]                     # [KD_H,128,BL,S]
        r_out[b0:b0 + BL] = o.transpose(2, 3, 0, 1).reshape(BL, S, H)
        hp = results[c]["hout"].reshape(128, KD_H, BL)
        hT[0, b0:b0 + BL] = hp.transpose(2, 1, 0).reshape(BL, H)
        cp = results[c]["cout"].reshape(128, KD_H, BL)
        cT[0, b0:b0 + BL] = cp.transpose(2, 1, 0).reshape(BL, H)
    return r_out, hT, cT


class _Res:
    results = None
    exec_time_ns = None
    summary = None
    neff_dir = None
    wall_exec_s = None


def _run_pjrt_timed(nc, in_maps, iters=2):
    """Like bass2jax.run_bass_via_pjrt (multi-core branch) but without
    donation, so the compiled executable can be re-run; returns
    (results, best_wall_seconds_of_warm_runs)."""
    import time as _time
    import jax
    from jax.sharding import Mesh, PartitionSpec
    from jax.experimental.shard_map import shard_map
    from concourse import bass2jax, mybir
    from concourse.bass2jax import _bass_exec_p, partition_id_tensor
    bass2jax.install_neuronx_cc_hook()

    partition_name = (nc.partition_id_tensor.name
                      if nc.partition_id_tensor else None)
    in_names, out_names, out_avals, zero_outs = [], [], [], []
    for alloc in nc.m.functions[0].allocations:
        if not isinstance(alloc, mybir.MemoryLocationSet):
            continue
        name = alloc.memorylocations[0].name
        if alloc.kind == "ExternalInput":
            if name != partition_name:
                in_names.append(name)
        elif alloc.kind == "ExternalOutput":
            out_names.append(name)
            shape = tuple(alloc.tensor_shape)
            dtype = mybir.dt.np(alloc.dtype)
            out_avals.append(jax.core.ShapedArray(shape, dtype))
            zero_outs.append(np.zeros(shape, dtype))
    n_params = len(in_names)
    in_names = in_names + out_names
    if partition_name is not None:
        in_names.append(partition_name)

    def _body(*args):
        operands = list(args)
        if partition_name is not None:
            operands.append(partition_id_tensor())
        outs = _bass_exec_p.bind(
            *operands, out_avals=tuple(out_avals), in_names=tuple(in_names),
            out_names=tuple(out_names), lowering_input_output_aliases=(),
            sim_require_finite=True, sim_require_nnan=True, nc=nc)
        return tuple(outs)

    n_cores = len(in_maps)
    devices = jax.devices()[:n_cores]
    mesh = Mesh(np.asarray(devices), ("core",))
    in_specs = (PartitionSpec("core"),) * (n_params + len(out_names))
    out_specs = (PartitionSpec("core"),) * len(out_names)
    sharded = jax.jit(shard_map(_body, mesh=mesh, in_specs=in_specs,
                                out_specs=out_specs, check_rep=False),
                      keep_unused=True)
    concat_in = [
        np.concatenate([np.asarray(in_maps[c][in_names[i]])
                        for c in range(n_cores)], axis=0)
        for i in range(n_params)
    ]
    concat_zeros = [np.zeros((n_cores * z.shape[0], *z.shape[1:]), z.dtype)
                    for z in zero_outs]
    args = concat_in + concat_zeros
    out_arrs = None
    best = None
    for it in range(max(1, iters)):
        t0 = _time.perf_counter()
        o = sharded(*args)
        jax.block_until_ready(o)
        dt = _time.perf_counter() - t0
        print(f"  exec iter {it}: {dt:.3f}s", flush=True)
        if it > 0:
            best = dt if best is None else min(best, dt)
        out_arrs = o
    results = [
        {name: np.asarray(out_arrs[i]).reshape(n_cores, *out_avals[i].shape)[c]
         for i, name in enumerate(out_names)}
        for c in range(n_cores)
    ]
    return results, best


def run(inputs, S=S_FULL, T=128, trace=False, timed=0):
    from concourse import bass2jax
    key = (S, T)
    if key not in _CACHE:
        _CACHE[key] = _build(S, T)
    nc = _CACHE[key]
    in_maps = _prep(inputs, S)

    res = _Res()
    if trace:
        import ntff_shim
        ntff_shim.install()
        from antenv.axon_hooks import get_axon_ntff_profile_hook
        import tempfile
        hook = get_axon_ntff_profile_hook()
        neff_dir = tempfile.mkdtemp(prefix="ntff_")
        res.neff_dir = neff_dir
        with hook(neff_dir, None):
            res.results = bass2jax.run_bass_via_pjrt(nc, in_maps, n_cores=NC)
        try:
            import gauge.profiler
            from concourse._compat import FishPath
            profile = gauge.profiler.Profile(
                profile_path=FishPath(neff_dir), kernel_dev_mode=True,
                profile_on_exit=False, bass_kernel=nc.m,
                offline_processing=True, fname="*_body*")
            profile.convert_ntffs_to_json((0,))
            data = profile.load_json(0)
            summ = data.get("summary") if isinstance(data, dict) else None
            if isinstance(summ, list):
                summ = summ[0] if summ else None
            res.summary = summ
            if summ and summ.get("total_time") is not None:
                res.exec_time_ns = int(float(summ["total_time"]) * 1e9)
        except Exception as e:
            import traceback
            traceback.print_exc()
            print("trace processing failed:", e, flush=True)
    elif timed:
        res.results, res.wall_exec_s = _run_pjrt_timed(nc, in_maps,
                                                       iters=timed)
    else:
        res.results = bass2jax.run_bass_via_pjrt(nc, in_maps, n_cores=NC)

    r_out, hT, cT = _gather(res.results, S)
    return (r_out, hT, cT), res


def kernel(**inputs):
    (r_out, hT, cT), _ = run(inputs)
    return r_out, hT, cT
